# revision 1
# baseline (speedup 1.0000x reference)
"""Bipartite GNN conv (variable->factor) Trainium2 kernel.

8 NeuronCores, no collectives:
  - FACTORS sharded by range (6250/core); each edge lives on the core owning
    its receiver; full output = concat of per-core outputs.
  - Host: receiver-sort edges; windows of 256 consecutive local factors;
    within a window, slots bucketed by sender bank (32768 rows -> int16).
  - Device per core:
      A = factors_local @ W_msg[:128] + b_msg   (f32, internal DRAM)
      gVT(hi/lo) = transposed bf16 dma_gather (<=512 idx, 4 SWDGE queues)
                   of variables_hilo rows -> [f, e] direct, ~f32 precision
      gA  = generic indirect DMA, one A-row per partition per edge tile
      m   = relu(gV @ W2 + gA)                  (PE matmuls, ACT relu)
      S[e,s] = (recv_rel[e] == s)               (DVE iota compare)
      aggrT[d,s] += m.T @ S                     (PE, PSUM accum per window)
      out = relu(aggr @ Wc2 + factors @ Wc1 + b_comb)   (f32)
"""

import os
import numpy as np

os.environ.setdefault("MYCRO_LOCAL_CACHE", "1")

D = 128
P = 128
NC = 8
WIN = 256          # factors per aggregation window
BANK = 32768       # variable rows per int16 gather bank
GCHUNK = 512       # max indices per dma_gather
M_BF16 = os.environ.get("GNN_M_DTYPE", "bf16") == "bf16"

_LAST_EXEC_NS = None
_LAST_RES = None
_TRACE = bool(int(os.environ.get("GNN_KERNEL_TRACE", "0")))


def _install_profile_shim():
    import sys
    import types
    import ctypes
    import contextlib

    try:
        import antenv
        try:
            from antenv.axon_hooks import get_axon_ntff_profile_hook  # noqa
        except ImportError:
            mod = types.ModuleType("antenv.axon_hooks")
            mod._hook = None
            mod.set_axon_ntff_profile_hook = lambda h: setattr(mod, "_hook", h)
            mod.get_axon_ntff_profile_hook = lambda: mod._hook
            sys.modules["antenv.axon_hooks"] = mod
            antenv.axon_hooks = mod

        from antenv.axon_hooks import (  # noqa
            get_axon_ntff_profile_hook, set_axon_ntff_profile_hook)
        if get_axon_ntff_profile_hook() is None:
            lib = ctypes.CDLL("/opt/axon/libaxon_pjrt.so")
            if hasattr(lib, "axon_start_nrt_profile"):
                lib.axon_start_nrt_profile.argtypes = [
                    ctypes.POINTER(ctypes.c_int64), ctypes.c_size_t]
                lib.axon_start_nrt_profile.restype = ctypes.c_int64
                lib.axon_stop_nrt_profile.argtypes = [ctypes.c_char_p]
                lib.axon_stop_nrt_profile.restype = ctypes.c_int64

                @contextlib.contextmanager
                def _hook(output_dir, device_ids):
                    import jax
                    jax.devices()
                    if device_ids:
                        ids = (ctypes.c_int64 * len(device_ids))(*device_ids)
                        rc = lib.axon_start_nrt_profile(ids, len(device_ids))
                    else:
                        rc = lib.axon_start_nrt_profile(None, 0)
                    if rc != 0:
                        raise RuntimeError(f"start_nrt_profile rc={rc}")
                    try:
                        yield
                    finally:
                        n = lib.axon_stop_nrt_profile(str(output_dir).encode())
                        print(f"profile: {n} file(s) -> {output_dir}",
                              file=sys.stderr)

                set_axon_ntff_profile_hook(_hook)

        import concourse.bass_utils as bu
        bu.upload_artifacts = lambda tmpdir: f"local:{tmpdir}"
    except Exception as e:
        print(f"profile shim failed: {e}", file=sys.stderr)


def _wrap16(lin):
    """Linear idx list -> dma_gather layout [128, n/16] (16-part wrap, 8x)."""
    blk = lin.reshape(-1, 16).T.copy()
    return np.tile(blk, (8, 1))


def _pack_inputs(variables, factors, senders, receivers, W_msg, b_msg, W_comb,
                 b_comb, n_cores=NC, bank=BANK, win=WIN):
    import ml_dtypes
    bf16 = ml_dtypes.bfloat16

    variables = np.ascontiguousarray(np.asarray(variables, dtype=np.float32))
    factors = np.ascontiguousarray(np.asarray(factors, dtype=np.float32))
    senders = np.asarray(senders).astype(np.int64)
    receivers = np.asarray(receivers).astype(np.int64)
    W_msg = np.asarray(W_msg, dtype=np.float32)
    b_msg = np.asarray(b_msg, dtype=np.float32).reshape(1, D)
    W_comb = np.asarray(W_comb, dtype=np.float32)
    b_comb = np.asarray(b_comb, dtype=np.float32).reshape(1, D)

    n_vars = variables.shape[0]
    n_factors = factors.shape[0]
    nb = (n_vars + bank - 1) // bank
    f_loc = n_factors // n_cores
    assert f_loc * n_cores == n_factors
    nw = (f_loc + win - 1) // win
    f_pad = nw * win

    # hi/lo split of variables (~f32 precision through two bf16 matmuls)
    v_hi = variables.astype(bf16)
    v_lo = (variables - v_hi.astype(np.float32)).astype(bf16)
    v_hilo = np.concatenate([v_hi, v_lo], axis=1)  # [V, 256] bf16
    W2 = np.ascontiguousarray(W_msg[D:])
    W2h = W2.astype(bf16)
    W2l = (W2 - W2h.astype(np.float32)).astype(bf16)

    order = np.argsort(receivers, kind="stable")
    rs = receivers[order]
    ss = senders[order]
    core_lo = np.searchsorted(rs, np.arange(n_cores) * f_loc)
    core_hi = np.searchsorted(rs, (np.arange(n_cores) + 1) * f_loc)

    counts = np.zeros((n_cores, nw, nb), np.int64)
    percore = []
    for c in range(n_cores):
        lo, hi = core_lo[c], core_hi[c]
        r_loc = (rs[lo:hi] - c * f_loc).astype(np.int64)
        s_gl = ss[lo:hi].astype(np.int64)
        w_of = r_loc // win
        b_of = s_gl // bank
        np.add.at(counts[c], (w_of, b_of), 1)
        percore.append((r_loc, s_gl, w_of, b_of))
    cap = np.maximum(counts.max(axis=(0, 1)), 1)
    Kb = ((cap + P - 1) // P).astype(np.int64)
    Cb = Kb * P
    K_tot = int(Kb.sum())
    t_off = np.concatenate([[0], np.cumsum(Kb)])

    in_maps = []
    for c in range(n_cores):
        r_loc, s_gl, w_of, b_of = percore[c]
        ordwb = np.lexsort((b_of, w_of))
        r_loc, s_gl, w_of, b_of = (r_loc[ordwb], s_gl[ordwb], w_of[ordwb],
                                   b_of[ordwb])
        cnt = counts[c]
        cum = np.zeros((nw, nb), np.int64)
        cum.flat[1:] = np.cumsum(cnt.flat)[:-1]
        j = np.arange(len(r_loc)) - cum[w_of, b_of]
        t_in_w = t_off[b_of] + j // P
        p = j % P

        # gV idx, per bank: [window][Cb[b]] linear; wrapped per window
        vidx_w = []
        for b in range(nb):
            arr = np.zeros((nw, Cb[b]), np.int16)
            m = b_of == b
            arr[w_of[m], j[m]] = (s_gl[m] - b * bank).astype(np.int16)
            vidx_w.append(np.concatenate(
                [_wrap16(arr[w]) for w in range(nw)], axis=0))

        # gA idx per tile: [nw, K_tot, 128] int32, absolute A rows, pad->0
        rabs = np.zeros((nw, K_tot, P), np.int32)
        rabs[w_of, t_in_w, p] = r_loc.astype(np.int32)
        # layout for one [128, K_tot] int32 DMA per window: [nw, 128, K_tot]
        rabs = np.ascontiguousarray(rabs.transpose(0, 2, 1))

        rrel = np.full((nw, P, K_tot), -1.0, np.float32)
        rrel[w_of, p, t_in_w] = (r_loc - w_of * win).astype(np.float32)

        floc = np.zeros((f_pad, D), np.float32)
        floc[:f_loc] = factors[c * f_loc:(c + 1) * f_loc]

        im = {
            "variables_hl": v_hilo,
            "factors_loc": floc,
            "rab_idx": rabs.reshape(nw * P, K_tot),
            "rrel": rrel.reshape(nw * P, K_tot),
            "W1": np.ascontiguousarray(W_msg[:D]),
            "W2h": W2h, "W2l": W2l,
            "Wc1": np.ascontiguousarray(W_comb[:D]),
            "Wc2": np.ascontiguousarray(W_comb[D:]),
            "bmsg": b_msg, "bcomb": b_comb,
            "ones_r": np.ones((1, D), np.float32),
            "iota_w": np.tile(np.arange(win, dtype=np.float32), (P, 1)),
            "ident": np.eye(P, dtype=np.float32),
        }
        for b in range(nb):
            im[f"vidx{b}"] = vidx_w[b]
        in_maps.append(im)

    params = dict(n_vars=n_vars, f_loc=f_loc, f_pad=f_pad, nw=nw, nb=nb,
                  Kb=[int(x) for x in Kb], K_tot=K_tot, n_cores=n_cores,
                  bank=bank, win=win)
    return in_maps, params


def _build_nc(params):
    import concourse.bacc as bacc
    import concourse.tile as tile
    import concourse.mybir as mybir
    from concourse import bass, library_config

    f32 = mybir.dt.float32
    bf16 = mybir.dt.bfloat16
    i16 = mybir.dt.int16
    i32 = mybir.dt.int32
    nv = params["n_vars"]
    nw, nb = params["nw"], params["nb"]
    Kb, K_tot = params["Kb"], params["K_tot"]
    f_pad, bank, win = params["f_pad"], params["bank"], params["win"]
    Cb = [k * P for k in Kb]
    nblk = f_pad // P
    relu_fn = mybir.ActivationFunctionType.Relu
    m_dt = bf16 if M_BF16 else f32

    nc = bacc.Bacc("TRN2", target_bir_lowering=False, debug=False,
                   num_swdge_queues=4)

    t_vars = nc.dram_tensor("variables_hl", [nv, 2 * D], bf16,
                            kind="ExternalInput")
    t_floc = nc.dram_tensor("factors_loc", [f_pad, D], f32,
                            kind="ExternalInput")
    t_rab = nc.dram_tensor("rab_idx", [nw * P, K_tot], i32,
                           kind="ExternalInput")
    t_rrel = nc.dram_tensor("rrel", [nw * P, K_tot], f32, kind="ExternalInput")
    t_vidx = [nc.dram_tensor(f"vidx{b}", [nw * P, Cb[b] // 16], i16,
                             kind="ExternalInput") for b in range(nb)]
    t_W1 = nc.dram_tensor("W1", [D, D], f32, kind="ExternalInput")
    t_W2h = nc.dram_tensor("W2h", [D, D], bf16, kind="ExternalInput")
    t_W2l = nc.dram_tensor("W2l", [D, D], bf16, kind="ExternalInput")
    t_Wc1 = nc.dram_tensor("Wc1", [D, D], f32, kind="ExternalInput")
    t_Wc2 = nc.dram_tensor("Wc2", [D, D], f32, kind="ExternalInput")
    t_bmsg = nc.dram_tensor("bmsg", [1, D], f32, kind="ExternalInput")
    t_bcomb = nc.dram_tensor("bcomb", [1, D], f32, kind="ExternalInput")
    t_ones = nc.dram_tensor("ones_r", [1, D], f32, kind="ExternalInput")
    t_iota = nc.dram_tensor("iota_w", [P, win], f32, kind="ExternalInput")
    t_id = nc.dram_tensor("ident", [P, P], f32, kind="ExternalInput")
    t_out = nc.dram_tensor("out", [f_pad, D], f32, kind="ExternalOutput")
    t_A = nc.dram_tensor("A_tab", [f_pad, D], f32)  # internal

    qn = [0]

    def next_q():
        qn[0] = (qn[0] + 1) % 4
        return qn[0]

    with tile.TileContext(nc) as tc:
        with (
            tc.tile_pool(name="const", bufs=1) as cpool,
            tc.tile_pool(name="ft", bufs=1) as ftpool,
            tc.tile_pool(name="io", bufs=3) as iopool,
            tc.tile_pool(name="gv", bufs=3) as gvpool,
            tc.tile_pool(name="ga", bufs=4) as gapool,
            tc.tile_pool(name="work", bufs=4) as wpool,
            tc.tile_pool(name="ps_t", bufs=2, space="PSUM") as ps_t,
            tc.tile_pool(name="ps_m", bufs=3, space="PSUM") as ps_m,
            tc.tile_pool(name="ps_agg", bufs=1, space="PSUM") as ps_agg,
            tc.tile_pool(name="ps_o", bufs=2, space="PSUM") as ps_o,
        ):
            nc.gpsimd.load_library(library_config.mlp)

            def cload(t, shape, dt):
                s = cpool.tile(shape, dt, tag=t.name)
                nc.sync.dma_start(out=s[:], in_=t[:])
                return s

            W1 = cload(t_W1, [D, D], f32)
            W2h = cload(t_W2h, [D, D], bf16)
            W2l = cload(t_W2l, [D, D], bf16)
            Wc1 = cload(t_Wc1, [D, D], f32)
            Wc2 = cload(t_Wc2, [D, D], f32)
            bmsg = cload(t_bmsg, [1, D], f32)
            bcomb = cload(t_bcomb, [1, D], f32)
            ones_r = cload(t_ones, [1, D], f32)
            iota = cload(t_iota, [P, win], f32)
            ident = cload(t_id, [P, P], f32)

            FT = ftpool.tile([P, f_pad], f32)

            # ---- precompute: FT + A = factors @ W1 + bmsg (f32, DRAM)
            for blk in range(nblk):
                rows = slice(blk * P, (blk + 1) * P)
                fl = iopool.tile([P, D], f32, tag="fload")
                nc.sync.dma_start(out=fl[:], in_=t_floc[rows, :])
                pt = ps_t.tile([P, P], f32)
                nc.tensor.transpose(out=pt[:], in_=fl[:], identity=ident[:])
                nc.vector.tensor_copy(out=FT[:, rows], in_=pt[:])
                pa = ps_o.tile([P, D], f32, tag="po")
                nc.tensor.matmul(pa[:], lhsT=ones_r[:1, :], rhs=bmsg[:1, :],
                                 start=True, stop=False)
                nc.tensor.matmul(pa[:], lhsT=FT[:, rows], rhs=W1[:],
                                 start=False, stop=True)
                asb = iopool.tile([P, D], f32, tag="astore")
                nc.scalar.copy(out=asb[:], in_=pa[:])
                nc.sync.dma_start(out=t_A[rows, :], in_=asb[:])

            # ---- edge phase
            for w in range(nw):
                wrow = slice(w * P, (w + 1) * P)
                # gV: transposed hi/lo gathers, <=512 idx each, own tile
                gv_tiles = {}   # bank -> list of (tile, nidx)
                for b in range(nb):
                    subs = []
                    off = 0
                    while off < Cb[b]:
                        n = min(GCHUNK, Cb[b] - off)
                        vix = iopool.tile([P, n // 16], i16,
                                          tag=f"vix{len(subs)}_{b}")
                        nc.sync.dma_start(
                            out=vix[:],
                            in_=t_vidx[b][wrow, off // 16:(off + n) // 16])
                        gt = gvpool.tile([P, 2 * n], bf16,
                                         tag=f"gvt{len(subs)}_{b}")
                        nc.gpsimd.dma_gather(
                            out_ap=gt[:].rearrange("p (c n) -> p c n", c=2),
                            in_ap=t_vars[b * bank:min((b + 1) * bank, nv), :],
                            idxs_ap=vix[:], num_idxs=n, num_idxs_reg=n,
                            elem_size=2 * D, transpose=True,
                            queue_num=next_q())
                        subs.append((gt, n))
                        off += n
                    gv_tiles[b] = subs

                ridx = iopool.tile([P, K_tot], i32, tag="ridx")
                nc.sync.dma_start(out=ridx[:], in_=t_rab[wrow, :])
                rrel = iopool.tile([P, K_tot], f32, tag="rrel")
                nc.sync.dma_start(out=rrel[:], in_=t_rrel[wrow, :])

                pagg = ps_agg.tile([P, win], f32)
                t_idx = 0
                for b in range(nb):
                    for kb in range(Kb[b]):
                        sub, loc = divmod(kb * P, GCHUNK)
                        gt, n = gv_tiles[b][sub]
                        hi = gt[:, loc:loc + P]
                        lo = gt[:, n + loc:n + loc + P]
                        ga = gapool.tile([P, D], f32, tag="ga")
                        nc.gpsimd.indirect_dma_start(
                            out=ga[:], out_offset=None, in_=t_A[:, :],
                            in_offset=bass.IndirectOffsetOnAxis(
                                ap=ridx[:, t_idx:t_idx + 1], axis=0))
                        pm = ps_m.tile([P, D], f32)
                        nc.tensor.matmul(pm[:], lhsT=hi, rhs=W2h[:],
                                         start=True, stop=False)
                        nc.tensor.matmul(pm[:], lhsT=lo, rhs=W2h[:],
                                         start=False, stop=False)
                        nc.tensor.matmul(pm[:], lhsT=hi, rhs=W2l[:],
                                         start=False, stop=False)
                        nc.tensor.matmul(pm[:], lhsT=ident[:], rhs=ga[:],
                                         start=False, stop=True)
                        msb = wpool.tile([P, D], m_dt, tag="msb")
                        nc.scalar.activation(msb[:], pm[:], relu_fn)
                        st = wpool.tile([P, win], m_dt, tag="st")
                        nc.vector.tensor_scalar(
                            out=st[:], in0=iota[:],
                            scalar1=rrel[:, t_idx:t_idx + 1],
                            scalar2=None, op0=mybir.AluOpType.is_equal)
                        nc.tensor.matmul(pagg[:], lhsT=msb[:], rhs=st[:],
                                         start=(t_idx == 0),
                                         stop=(t_idx == K_tot - 1))
                        t_idx += 1

                aggT = wpool.tile([P, win], f32, tag="aggT")
                nc.vector.tensor_copy(out=aggT[:], in_=pagg[:])
                for h in range(win // P):
                    po = ps_o.tile([P, D], f32, tag="po")
                    nc.tensor.matmul(po[:], lhsT=ones_r[:1, :],
                                     rhs=bcomb[:1, :], start=True, stop=False)
                    nc.tensor.matmul(po[:], lhsT=aggT[:, h * P:(h + 1) * P],
                                     rhs=Wc2[:], start=False, stop=False)
                    fcol = w * win + h * P
                    nc.tensor.matmul(po[:], lhsT=FT[:, fcol:fcol + P],
                                     rhs=Wc1[:], start=False, stop=True)
                    osb = iopool.tile([P, D], f32, tag="osb")
                    nc.scalar.activation(osb[:], po[:], relu_fn)
                    nc.sync.dma_start(out=t_out[fcol:fcol + P, :], in_=osb[:])

    nc.compile()
    return nc


def kernel(**inputs):
    global _LAST_EXEC_NS, _LAST_RES
    from concourse.bass_utils import run_bass_kernel_spmd

    in_maps, params = _pack_inputs(**inputs)
    n_cores = params["n_cores"]
    nc = _build_nc(params)
    if _TRACE:
        _install_profile_shim()
        try:
            res = run_bass_kernel_spmd(nc, in_maps, list(range(n_cores)),
                                       trace=True, tmpdir=os.environ.get(
                                           "GNN_KERNEL_TRACE_DIR"))
        except Exception as e:
            import sys
            print(f"traced run failed ({e}); retrying untraced",
                  file=sys.stderr)
            res = run_bass_kernel_spmd(nc, in_maps, list(range(n_cores)))
    else:
        res = run_bass_kernel_spmd(nc, in_maps, list(range(n_cores)))
    _LAST_EXEC_NS = res.exec_time_ns
    _LAST_RES = res
    f_loc = params["f_loc"]
    out = np.concatenate([res.results[c]["out"][:f_loc]
                          for c in range(n_cores)], axis=0)
    return out.astype(np.float32)

